# Initial kernel scaffold
#
"""CRF token-mean loss (forward-algorithm denominator + gold-path numerator)
for Trainium2, data-parallel over 8 NeuronCores (batch sharding).

Full inputs in, full (scalar) output out. Per core: 128 sequences x L=1024
steps x T=21 tags.

Denominator: multiplicative-domain forward scan  p <- (E^T p) * x_l  with
E = exp(transitions), x_l = exp(emissions_l), periodic renormalization
(every RENORM_W steps) with the log-offsets accumulated separately.

Numerator: one-hot mask (is_equal vs iota) used three ways:
  - fused mask*emissions multiply-accumulate (gold emission score)
  - one-hot Gram matmuls accumulating transition-pair counts in PSUM,
    then counts . transitions
  - start/end row gathers at l=0 / l=L-1.
"""

import numpy as np
import ml_dtypes

import concourse.bass as bass
import concourse.tile as tile
from concourse import bacc, mybir
from concourse.bass_utils import run_bass_kernel_spmd

F32 = mybir.dt.float32
BF16 = mybir.dt.bfloat16
I32 = mybir.dt.int32
U8 = mybir.dt.uint8

ALU = mybir.AluOpType
ACTF = mybir.ActivationFunctionType

N_CORES = 8
B, L, T = 1024, 1024, 21
BLOC = B // N_CORES          # 128 sequences per core
TPAD = 32                    # t padded to 32 partitions per l in transposed x
LCHUNK = 128                 # emissions l-chunk per DMA/mask pass
NCHUNK = L // LCHUNK
XPAGE_L = 16                 # l steps per transposed-x page tile
RENORM_W = 16                # renormalize every W scan steps
N_TRACKS = 2                 # independent scan chains (column split)


# byte offsets inside the packed per-partition constant blob
OFF_TRANS = 0          # f32 [21 rows meaningful, 21]
OFF_STARTREP = 84      # f32 [128, 21]
OFF_ENDREP = 168       # f32 [128, 21]
OFF_ESTART = 252       # f32 [128, 1] (rows 0..20)
OFF_EEND = 256         # f32 [128, 1]
OFF_ONESF = 260        # f32 [128, 1] all ones
OFF_ONESROW = 264      # f32 [128, 21] all ones
OFF_ETRANS = 348       # bf16 [21 rows, 21]
OFF_ONESB = 390        # bf16 [128, 1] all ones
OFF_IOTA = 392         # i32 [128, 21]
OFF_TAGS = 480         # i32 [128, 1024]
OFF_MASK = 4576        # u8 [128, 1024]
OFF_IDENT = 5600       # f32 [128, 128] identity
BLOB_BYTES = 6144


DEBUG = False
REPS = 1
SKIP_SCAN = False
SKIP_NUM = False


def _build(nc):
    em_d = nc.dram_tensor("em", [BLOC, L, T], F32, kind="ExternalInput").ap()
    blob_d = nc.dram_tensor("blob", [128, BLOB_BYTES], U8,
                            kind="ExternalInput").ap()
    out_d = nc.dram_tensor("out", [1, 8], F32, kind="ExternalOutput").ap()
    if DEBUG:
        xdbg_d = nc.dram_tensor("xdbg", [128, 512], F32,
                                kind="ExternalOutput").ap()
        pdbg_d = nc.dram_tensor("pdbg", [T, 512], F32,
                                kind="ExternalOutput").ap()
        ddbg_d = nc.dram_tensor("ddbg", [1, 8448], F32,
                                kind="ExternalOutput").ap()

    with tile.TileContext(nc) as tc:
        with (
            tc.tile_pool(name="singles", bufs=1) as singles,
            tc.tile_pool(name="embuf", bufs=2) as embuf,
            tc.tile_pool(name="maskbuf", bufs=2) as maskbuf,
            tc.tile_pool(name="xbuf", bufs=1) as xbuf,
            tc.tile_pool(name="state", bufs=1) as state,
            tc.tile_pool(name="small", bufs=4) as small,
            tc.tile_pool(name="ps_tr", bufs=2, space="PSUM") as ps_tr,
            tc.tile_pool(name="ps_q", bufs=2, space="PSUM") as ps_q,
            tc.tile_pool(name="ps_misc", bufs=2, space="PSUM") as ps_misc,
            tc.tile_pool(name="ps_gram", bufs=1, space="PSUM") as ps_gram,
        ):
            # ---- load all constants/params/tags/mask in ONE DMA ----
            blob = singles.tile([128, BLOB_BYTES], U8)
            nc.sync.dma_start(out=blob, in_=blob_d)

            def fview(off, n):
                return blob[:, off:off + 4 * n].bitcast(F32)

            trans = fview(OFF_TRANS, T)[0:T, :]
            startrep = fview(OFF_STARTREP, T)
            endrep = fview(OFF_ENDREP, T)
            estart = fview(OFF_ESTART, 1)[0:T, :]
            eend = fview(OFF_EEND, 1)[0:T, :]
            ones128 = fview(OFF_ONESF, 1)
            ones21f = fview(OFF_ONESF, 1)[0:T, :]
            ones1x21 = fview(OFF_ONESROW, T)[0:1, :]
            ident = fview(OFF_IDENT, 128)
            etrans = blob[:, OFF_ETRANS:OFF_ETRANS + 2 * T].bitcast(BF16)[0:T, :]
            ones21b = blob[:, OFF_ONESB:OFF_ONESB + 2].bitcast(BF16)[0:T, :]
            iota = blob[:, OFF_IOTA:OFF_IOTA + 4 * T].bitcast(I32)
            tags_sb = blob[:, OFF_TAGS:OFF_TAGS + 4 * L].bitcast(I32)
            mask_sb = blob[:, OFF_MASK:OFF_MASK + L]

            # accumulators
            em_acc = singles.tile([BLOC, NCHUNK], F32)     # gold-emission partial
            se_acc = singles.tile([BLOC, 2], F32)          # start/end partials
            lnbuf = singles.tile([1, 128, L // RENORM_W], F32)  # renorm logs
            out_sb = singles.tile([1, 8], F32)
            nc.vector.memset(out_sb, 0.0)

            # resident transposed x pages: partitions (l%4)*32 + t,
            # columns ((l % XPAGE_L) // 4)*128 + b
            NPAGES = L // XPAGE_L
            xpages = [xbuf.tile([128, (XPAGE_L // 4) * 128], BF16, tag=f"xp{i}",
                                name=f"xp{i}") for i in range(NPAGES)]

            def x_slice(l, c0, c1):
                pg = xpages[l // XPAGE_L]
                pb = (l % 4) * 32
                cb = ((l % XPAGE_L) // 4) * 128
                return pg[pb:pb + T, cb + c0:cb + c1]

            for rep in range(REPS):
                # mask tiles per chunk are needed at chunk boundaries for the
                # gram matmuls; keep per-chunk handles
                mask_tiles = []

                for c in range(NCHUNK):
                    # em in 32-padded layout [128, Lc, 32]; pad lanes carry
                    # garbage that only ever reaches unread psum partitions
                    em_t = embuf.tile([BLOC, LCHUNK * TPAD], F32, tag="em",
                                      name="em_t")
                    dst = bass.AP(
                        tensor=em_t.tensor, offset=em_t.offset,
                        ap=[em_t.ap[0], [TPAD, LCHUNK], [1, T]],
                    )
                    nc.sync.dma_start(out=dst, in_=em_d[:, c * LCHUNK:(c + 1) * LCHUNK, :])

                    # ---- one-hot mask for this chunk (bf16) ----
                    mk = maskbuf.tile([BLOC, LCHUNK, T], BF16, tag="mk")
                    tags_b = bass.AP(
                        tensor=tags_sb.tensor, offset=tags_sb.offset + c * LCHUNK,
                        ap=[tags_sb.ap[0], [1, LCHUNK], [0, T]],
                    )
                    iota_b = bass.AP(
                        tensor=iota.tensor, offset=iota.offset,
                        ap=[iota.ap[0], [0, LCHUNK], [1, T]],
                    )
                    if not SKIP_NUM:
                        nc.vector.tensor_tensor(out=mk, in0=tags_b, in1=iota_b,
                                                op=ALU.is_equal)
                    mask_tiles.append(mk)

                    # ---- gold emission score: accum(mask * em) ----
                    em_v = bass.AP(
                        tensor=em_t.tensor, offset=em_t.offset,
                        ap=[em_t.ap[0], [TPAD, LCHUNK], [1, T]],
                    )
                    mk_v = bass.AP(
                        tensor=mk.tensor, offset=mk.offset,
                        ap=[mk.ap[0], [T, LCHUNK], [1, T]],
                    )
                    scr = maskbuf.tile([BLOC, LCHUNK * T], BF16, tag="scr", name="scr")
                    if not SKIP_NUM:
                        nc.vector.scalar_tensor_tensor(
                        out=scr, in0=mk_v, scalar=1.0, in1=em_v,
                        op0=ALU.mult, op1=ALU.mult,
                        accum_out=em_acc[:, c:c + 1],
                    )

                    # ---- start / end gathers ----
                    if c == 0 and not SKIP_NUM:
                        nc.vector.scalar_tensor_tensor(
                            out=small.tile([BLOC, T], F32, tag="seg", name="seg"),
                            in0=mk[:, 0, :], scalar=1.0, in1=startrep,
                            op0=ALU.mult, op1=ALU.mult,
                            accum_out=se_acc[:, 0:1],
                        )
                    if c == NCHUNK - 1 and not SKIP_NUM:
                        nc.vector.scalar_tensor_tensor(
                            out=small.tile([BLOC, T], F32, tag="seg", name="seg"),
                            in0=mk[:, LCHUNK - 1, :], scalar=1.0, in1=endrep,
                            op0=ALU.mult, op1=ALU.mult,
                            accum_out=se_acc[:, 1:2],
                        )

                    # ---- transition-count gram matmuls (PSUM accumulate) ----
                    # C[i,j] += sum_b onehot_l[b,i] * onehot_{l+1}[b,j]
                    if c == 0:
                        gram = ps_gram.tile([T, T], F32, name="gram")
                    for l in (range(LCHUNK) if not SKIP_NUM else []):
                        gl = c * LCHUNK + l
                        if gl >= L - 1:
                            continue
                        lhsT = mk[:, l, :]
                        if l + 1 < LCHUNK:
                            rhs = mk[:, l + 1, :]
                        else:
                            rhs = None  # handled by next chunk's l=0 vs prev
                        if rhs is not None:
                            nc.tensor.matmul(
                                out=gram, lhsT=lhsT, rhs=rhs,
                                start=(gl == 0), stop=(gl == L - 2),
                                skip_group_check=True,
                            )
                    if c > 0 and not SKIP_NUM:
                        # boundary pair (prev chunk last l, this chunk l=0)
                        nc.tensor.matmul(
                            out=gram, lhsT=mask_tiles[c - 1][:, LCHUNK - 1, :],
                            rhs=mk[:, 0, :],
                            start=False, stop=False,
                            skip_group_check=True,
                        )

                    # ---- transpose em -> psum, exp-drain -> x pages (bf16) ----
                    # blocks of 4 l (32-padded t): in [128b, (4l,32t)] -> out
                    # [(4l*32t), 128b]
                    for blk in range(LCHUNK // 4):
                        l0 = c * LCHUNK + blk * 4
                        src = bass.AP(
                            tensor=em_t.tensor,
                            offset=em_t.offset + blk * 4 * TPAD,
                            ap=[em_t.ap[0], [1, 4 * TPAD]],
                        )
                        pg = l0 // XPAGE_L
                        col = ((l0 % XPAGE_L) // 4) * 128
                        if (l0 % XPAGE_L) == 0:
                            ps_x = ps_tr.tile([128, (XPAGE_L // 4) * 128], F32,
                                              tag="psx")
                        nc.tensor.transpose(
                            out=ps_x[:, col:col + 128], in_=src, identity=ident,
                        )
                        if (l0 % XPAGE_L) == XPAGE_L - 4:
                            nc.scalar.activation(
                                out=xpages[pg], in_=ps_x, func=ACTF.Exp,
                            )

                # ---- transition score: counts . trans ----
                tacc = small.tile([T, 1], F32, tag="tacc")
                if SKIP_NUM:
                    nc.vector.memset(tacc, 0.0)
                else:
                    nc.vector.scalar_tensor_tensor(
                    out=small.tile([T, T], F32, tag="tscr", name="tscr"),
                    in0=gram, scalar=1.0, in1=trans,
                    op0=ALU.mult, op1=ALU.mult,
                    accum_out=tacc,
                )

                # ---- masksum ----
                msum = small.tile([BLOC, 1], F32, tag="msum")
                nc.vector.tensor_reduce(out=msum, in_=mask_sb,
                                        axis=mybir.AxisListType.XYZW, op=ALU.add)

                # ================= forward scan =================
                TRW = 128 // N_TRACKS
                p = state.tile([T, 128], BF16)
                # p0 = x_0 * exp(start)
                nc.vector.tensor_scalar(
                    out=p, in0=x_slice(0, 0, 128), scalar1=estart, scalar2=None,
                    op0=ALU.mult,
                )
                if DEBUG:
                    pdbg = singles.tile([T, 512], F32)
                    nc.vector.tensor_copy(out=pdbg[:, 0:128], in_=p)
                C_idx = [0] * N_TRACKS
                for l in (range(1, L) if not SKIP_SCAN else []):
                    for tr in range(N_TRACKS):
                        c0, c1 = tr * TRW, (tr + 1) * TRW
                        q = ps_q.tile([T, TRW], F32, tag="q", name="q")
                        nc.tensor.matmul(out=q, lhsT=etrans, rhs=p[:, c0:c1],
                                         start=True, stop=True)
                        nc.vector.tensor_tensor(
                            out=p[:, c0:c1], in0=q, in1=x_slice(l, c0, c1),
                            op=ALU.mult,
                        )
                        # renorm (staggered across tracks)
                        if l % RENORM_W == (RENORM_W // 2) * tr % RENORM_W and l > 0:
                            s = ps_misc.tile([1, TRW], F32, tag="misc", name="s")
                            nc.tensor.matmul(out=s, lhsT=ones21b, rhs=p[:, c0:c1],
                                             start=True, stop=True)
                            r = small.tile([1, TRW], F32, tag="r", name="r")
                            nc.vector.reciprocal(out=r, in_=s)
                            rb = ps_misc.tile([T, TRW], F32, tag="misc", name="rb")
                            nc.tensor.matmul(out=rb, lhsT=ones1x21, rhs=r,
                                             start=True, stop=True)
                            nc.vector.tensor_tensor(out=p[:, c0:c1], in0=p[:, c0:c1],
                                                    in1=rb, op=ALU.mult)
                            # ln(s) into the deferred log buffer
                            ev = C_idx[tr]
                            C_idx[tr] += 1
                            # ACT Ln is only exact for inputs < 2^64: feed
                            # s * 2^-40 and add the 40*ln2 back on the host
                            nc.scalar.activation(
                                out=lnbuf[:, c0:c1, ev], in_=s, func=ACTF.Ln,
                                scale=2.0 ** -40,
                            )
                    if DEBUG and l <= 3:
                        nc.vector.tensor_copy(out=pdbg[:, 128 * l:128 * (l + 1)],
                                              in_=p)

                if DEBUG:
                    xcp = singles.tile([128, 512], F32)
                    nc.vector.tensor_copy(out=xcp, in_=xpages[0])
                    nc.sync.dma_start(out=xdbg_d, in_=xcp)
                    nc.sync.dma_start(out=pdbg_d, in_=pdbg)

            # ---- finalize denominator ----
            pf = small.tile([T, 128], F32, tag="pf")
            nc.vector.tensor_scalar(out=pf, in0=p, scalar1=eend, scalar2=None,
                                    op0=ALU.mult)
            zf = ps_misc.tile([1, 128], F32, tag="misc", name="zf")
            nc.tensor.matmul(out=zf, lhsT=ones21f, rhs=pf, start=True, stop=True)
            lnz = small.tile([1, 128], F32, tag="lnz")
            nc.scalar.activation(out=lnz, in_=zf, func=ACTF.Ln,
                                 scale=2.0 ** -40)
            # pad unused renorm-event slots with zeros
            nev = L // RENORM_W
            for tr in range(N_TRACKS):
                c0, c1 = tr * TRW, (tr + 1) * TRW
                if C_idx[tr] < nev:
                    nc.vector.memset(lnbuf[:, c0:c1, C_idx[tr]:nev], 0.0)
            csum = small.tile([1, 128], F32, tag="csum")
            nc.vector.tensor_reduce(out=csum, in_=lnbuf,
                                    axis=mybir.AxisListType.X, op=ALU.add)
            denomv = small.tile([1, 128], F32, tag="denomv")
            nc.vector.tensor_tensor(out=denomv, in0=lnz, in1=csum, op=ALU.add)
            dsum = small.tile([1, 1], F32, tag="dsum")
            nc.vector.tensor_reduce(out=dsum, in_=denomv,
                                    axis=mybir.AxisListType.XYZW, op=ALU.add)
            if DEBUG:
                ddbg = singles.tile([1, 8448], F32)
                nc.vector.tensor_copy(out=ddbg[:, 0:8192],
                                      in_=lnbuf.rearrange("p a b -> p (a b)"))
                nc.vector.tensor_copy(out=ddbg[:, 8192:8320], in_=lnz)
                nc.vector.tensor_copy(out=ddbg[:, 8320:8448], in_=csum)
                nc.sync.dma_start(out=ddbg_d, in_=ddbg)

            # ---- gather partials: [128, 4] -> ones-matmul -> [1, 4] ----
            parts = small.tile([BLOC, 4], F32, tag="parts")
            nc.vector.tensor_reduce(out=parts[:, 0:1], in_=em_acc,
                                    axis=mybir.AxisListType.XYZW, op=ALU.add)
            nc.vector.tensor_reduce(out=parts[:, 1:2], in_=se_acc,
                                    axis=mybir.AxisListType.XYZW, op=ALU.add)
            nc.vector.tensor_copy(out=parts[:, 2:3], in_=msum)
            nc.vector.memset(parts[:, 3:4], 0.0)
            psum4 = ps_misc.tile([1, 4], F32, tag="misc", name="psum4")
            nc.tensor.matmul(out=psum4, lhsT=ones128, rhs=parts,
                             start=True, stop=True)
            tsum = ps_misc.tile([1, 1], F32, tag="misc", name="tsum")
            nc.tensor.matmul(out=tsum, lhsT=ones21f, rhs=tacc,
                             start=True, stop=True)

            nc.vector.tensor_copy(out=out_sb[:, 0:4], in_=psum4)
            nc.vector.tensor_copy(out=out_sb[:, 4:5], in_=tsum)
            nc.vector.tensor_copy(out=out_sb[:, 5:6], in_=dsum)
            nc.sync.dma_start(out=out_d, in_=out_sb)

    return nc


_NC_CACHE = None


def _get_nc():
    global _NC_CACHE
    if _NC_CACHE is None:
        nc = bacc.Bacc("TRN2", target_bir_lowering=False, debug=False,
                       enable_asserts=False, num_devices=N_CORES)
        _build(nc)
        nc.compile()
        _NC_CACHE = nc
    return _NC_CACHE


def kernel(emissions, tags, mask, start_transitions, end_transitions,
           transitions):
    em = np.ascontiguousarray(np.asarray(emissions, dtype=np.float32))
    tg = np.ascontiguousarray(np.asarray(tags).astype(np.int32))
    mk = np.ascontiguousarray(np.asarray(mask).astype(np.uint8))
    start = np.asarray(start_transitions, dtype=np.float32)
    end = np.asarray(end_transitions, dtype=np.float32)
    trans = np.ascontiguousarray(np.asarray(transitions, dtype=np.float32))

    etrans = np.exp(trans.astype(np.float64)).astype(ml_dtypes.bfloat16)
    estart = np.exp(start.astype(np.float64)).astype(np.float32)
    eend = np.exp(end.astype(np.float64)).astype(np.float32)

    def pack_blob(tg_sh, mk_sh):
        blob = np.zeros((128, BLOB_BYTES), np.uint8)

        def put(off, arr2d):
            a = np.ascontiguousarray(arr2d)
            b = a.view(np.uint8).reshape(a.shape[0], -1)
            blob[:b.shape[0], off:off + b.shape[1]] = b

        put(OFF_TRANS, trans)
        put(OFF_STARTREP, np.broadcast_to(start, (128, T)))
        put(OFF_ENDREP, np.broadcast_to(end, (128, T)))
        put(OFF_ESTART, np.pad(estart.reshape(T, 1), ((0, 107), (0, 0))))
        put(OFF_EEND, np.pad(eend.reshape(T, 1), ((0, 107), (0, 0))))
        put(OFF_ONESF, np.ones((128, 1), np.float32))
        put(OFF_ONESROW, np.ones((128, T), np.float32))
        put(OFF_ETRANS, etrans)
        put(OFF_ONESB, np.ones((128, 1), ml_dtypes.bfloat16))
        put(OFF_IOTA, np.broadcast_to(np.arange(T, dtype=np.int32), (128, T)))
        put(OFF_TAGS, tg_sh)
        put(OFF_MASK, mk_sh)
        put(OFF_IDENT, np.eye(128, dtype=np.float32))
        return blob

    in_maps = []
    for c in range(N_CORES):
        sl = slice(c * BLOC, (c + 1) * BLOC)
        in_maps.append(dict(em=em[sl], blob=pack_blob(tg[sl], mk[sl])))

    nc = _get_nc()
    global _last_in_maps, _last_results
    _last_in_maps = in_maps
    res = run_bass_kernel_spmd(nc, in_maps, core_ids=list(range(N_CORES)))
    _last_results = res.results

    score = 0.0
    denom = 0.0
    masksum = 0.0
    # every Ln on device was fed s * 2^-40; add the offsets back:
    # per core, track0 columns have 63 renorm events, track1 have 64,
    # plus one final logsumexp per sequence column.
    ln_corr = (64 * 63 + 64 * 64 + 128) * 40.0 * np.log(2.0)
    for r in res.results:
        o = r["out"].astype(np.float64).ravel()
        score += o[0] + o[1] + o[4]   # emission + start/end + transition
        denom += o[5] + ln_corr
        masksum += o[2]
    return np.float32((score - denom) / masksum)



# revision 1
# speedup vs baseline: 80.4580x; 80.4580x over previous
"""CRF token-mean loss (forward-algorithm denominator + gold-path numerator)
for Trainium2, data-parallel over 8 NeuronCores (batch sharding).

Full inputs in, full (scalar) output out. Per core: 128 sequences x L=1024
steps x T=21 tags.

Denominator: multiplicative-domain forward scan  p <- (E^T p) * x_l  with
E = exp(transitions), x_l = exp(emissions_l), periodic renormalization
(every RENORM_W steps) with the log-offsets accumulated separately.

Numerator: one-hot mask (is_equal vs iota) used three ways:
  - fused mask*emissions multiply-accumulate (gold emission score)
  - one-hot Gram matmuls accumulating transition-pair counts in PSUM,
    then counts . transitions
  - start/end row gathers at l=0 / l=L-1.
"""

import numpy as np
import ml_dtypes

import concourse.bass as bass
import concourse.tile as tile
from concourse import bacc, mybir
from concourse.bass_utils import run_bass_kernel_spmd

F32 = mybir.dt.float32
BF16 = mybir.dt.bfloat16
I32 = mybir.dt.int32
U8 = mybir.dt.uint8

ALU = mybir.AluOpType
ACTF = mybir.ActivationFunctionType

N_CORES = 8
B, L, T = 1024, 1024, 21
BLOC = B // N_CORES          # 128 sequences per core
TPAD = 32                    # t padded to 32 partitions per l in transposed x
LCHUNK = 128                 # emissions l-chunk per DMA/mask pass
NCHUNK = L // LCHUNK
XPAGE_L = 16                 # l steps per transposed-x page tile
RENORM_W = 16                # renormalize every W scan steps
N_TRACKS = 2                 # independent scan chains (column split)


# byte offsets inside the packed per-partition constant blob
OFF_TRANS = 0          # f32 [21 rows meaningful, 21]
OFF_STARTREP = 84      # f32 [128, 21]
OFF_ENDREP = 168       # f32 [128, 21]
OFF_ESTART = 252       # f32 [128, 1] (rows 0..20)
OFF_EEND = 256         # f32 [128, 1]
OFF_ONESF = 260        # f32 [128, 1] all ones
OFF_ONESROW = 264      # f32 [128, 21] all ones
OFF_ETRANS = 348       # bf16 [21 rows, 21]
OFF_ONESB = 390        # bf16 [128, 1] all ones
OFF_IOTA = 392         # i32 [128, 21]
OFF_TAGS = 480         # i32 [128, 1024]
OFF_MASK = 4576        # u8 [128, 1024]
OFF_IDENT = 5600       # f32 [128, 128] identity
BLOB_BYTES = 6144


DEBUG = False
REPS = 1
SKIP_SCAN = False
SKIP_NUM = False


def _build(nc):
    em_d = nc.dram_tensor("em", [BLOC, L, T], F32, kind="ExternalInput").ap()
    blob_d = nc.dram_tensor("blob", [128, BLOB_BYTES], U8,
                            kind="ExternalInput").ap()
    out_d = nc.dram_tensor("out", [1, 8], F32, kind="ExternalOutput").ap()
    if DEBUG:
        xdbg_d = nc.dram_tensor("xdbg", [128, 512], F32,
                                kind="ExternalOutput").ap()
        pdbg_d = nc.dram_tensor("pdbg", [T, 512], F32,
                                kind="ExternalOutput").ap()
        ddbg_d = nc.dram_tensor("ddbg", [1, 8448], F32,
                                kind="ExternalOutput").ap()

    with tile.TileContext(nc) as tc:
        with (
            tc.tile_pool(name="singles", bufs=1) as singles,
            tc.tile_pool(name="embuf", bufs=2) as embuf,
            tc.tile_pool(name="maskbuf", bufs=2) as maskbuf,
            tc.tile_pool(name="xbuf", bufs=1) as xbuf,
            tc.tile_pool(name="state", bufs=1) as state,
            tc.tile_pool(name="small", bufs=4) as small,
            tc.tile_pool(name="ps_tr", bufs=2, space="PSUM") as ps_tr,
            tc.tile_pool(name="ps_q", bufs=2, space="PSUM") as ps_q,
            tc.tile_pool(name="ps_misc", bufs=2, space="PSUM") as ps_misc,
            tc.tile_pool(name="ps_gram", bufs=1, space="PSUM") as ps_gram,
        ):
            # ---- load all constants/params/tags/mask in ONE DMA ----
            blob = singles.tile([128, BLOB_BYTES], U8)
            nc.sync.dma_start(out=blob, in_=blob_d)

            def fview(off, n):
                return blob[:, off:off + 4 * n].bitcast(F32)

            trans = fview(OFF_TRANS, T)[0:T, :]
            startrep = fview(OFF_STARTREP, T)
            endrep = fview(OFF_ENDREP, T)
            estart = fview(OFF_ESTART, 1)[0:T, :]
            eend = fview(OFF_EEND, 1)[0:T, :]
            ones128 = fview(OFF_ONESF, 1)
            ones21f = fview(OFF_ONESF, 1)[0:T, :]
            ones1x21 = fview(OFF_ONESROW, T)[0:1, :]
            ident = fview(OFF_IDENT, 128)
            etrans = blob[:, OFF_ETRANS:OFF_ETRANS + 2 * T].bitcast(BF16)[0:T, :]
            ones21b = blob[:, OFF_ONESB:OFF_ONESB + 2].bitcast(BF16)[0:T, :]
            iota = blob[:, OFF_IOTA:OFF_IOTA + 4 * T].bitcast(I32)
            tags_sb = blob[:, OFF_TAGS:OFF_TAGS + 4 * L].bitcast(I32)
            mask_sb = blob[:, OFF_MASK:OFF_MASK + L]

            # accumulators
            em_acc = singles.tile([BLOC, NCHUNK], F32)     # gold-emission partial
            se_acc = singles.tile([BLOC, 2], F32)          # start/end partials
            lnbuf = singles.tile([1, 128, L // RENORM_W], F32)  # renorm logs
            out_sb = singles.tile([1, 8], F32)
            nc.vector.memset(out_sb, 0.0)

            # resident transposed x pages: partitions (l%4)*32 + t,
            # columns ((l % XPAGE_L) // 4)*128 + b
            NPAGES = L // XPAGE_L
            xpages = [xbuf.tile([128, (XPAGE_L // 4) * 128], BF16, tag=f"xp{i}",
                                name=f"xp{i}") for i in range(NPAGES)]

            def x_slice(l, c0, c1):
                pg = xpages[l // XPAGE_L]
                pb = (l % 4) * 32
                cb = ((l % XPAGE_L) // 4) * 128
                return pg[pb:pb + T, cb + c0:cb + c1]

            for rep in range(REPS):
                # mask tiles per chunk are needed at chunk boundaries for the
                # gram matmuls; keep per-chunk handles
                mask_tiles = []

                for c in range(NCHUNK):
                    # em in 32-padded layout [128, Lc, 32]; pad lanes carry
                    # garbage that only ever reaches unread psum partitions
                    em_t = embuf.tile([BLOC, LCHUNK * TPAD], F32, tag="em",
                                      name="em_t")
                    dst = bass.AP(
                        tensor=em_t.tensor, offset=em_t.offset,
                        ap=[em_t.ap[0], [TPAD, LCHUNK], [1, T]],
                    )
                    nc.sync.dma_start(out=dst, in_=em_d[:, c * LCHUNK:(c + 1) * LCHUNK, :])

                    # ---- one-hot mask for this chunk (bf16) ----
                    mk = maskbuf.tile([BLOC, LCHUNK, T], BF16, tag="mk")
                    tags_b = bass.AP(
                        tensor=tags_sb.tensor, offset=tags_sb.offset + c * LCHUNK,
                        ap=[tags_sb.ap[0], [1, LCHUNK], [0, T]],
                    )
                    iota_b = bass.AP(
                        tensor=iota.tensor, offset=iota.offset,
                        ap=[iota.ap[0], [0, LCHUNK], [1, T]],
                    )
                    if not SKIP_NUM:
                        nc.vector.tensor_tensor(out=mk, in0=tags_b, in1=iota_b,
                                                op=ALU.is_equal)
                    mask_tiles.append(mk)

                    # ---- gold emission score: accum(mask * em) ----
                    em_v = bass.AP(
                        tensor=em_t.tensor, offset=em_t.offset,
                        ap=[em_t.ap[0], [TPAD, LCHUNK], [1, T]],
                    )
                    mk_v = bass.AP(
                        tensor=mk.tensor, offset=mk.offset,
                        ap=[mk.ap[0], [T, LCHUNK], [1, T]],
                    )
                    scr = maskbuf.tile([BLOC, LCHUNK * T], BF16, tag="scr", name="scr")
                    if not SKIP_NUM:
                        nc.vector.scalar_tensor_tensor(
                        out=scr, in0=mk_v, scalar=1.0, in1=em_v,
                        op0=ALU.mult, op1=ALU.mult,
                        accum_out=em_acc[:, c:c + 1],
                    )

                    # ---- start / end gathers ----
                    if c == 0 and not SKIP_NUM:
                        nc.vector.scalar_tensor_tensor(
                            out=small.tile([BLOC, T], F32, tag="seg", name="seg"),
                            in0=mk[:, 0, :], scalar=1.0, in1=startrep,
                            op0=ALU.mult, op1=ALU.mult,
                            accum_out=se_acc[:, 0:1],
                        )
                    if c == NCHUNK - 1 and not SKIP_NUM:
                        nc.vector.scalar_tensor_tensor(
                            out=small.tile([BLOC, T], F32, tag="seg", name="seg"),
                            in0=mk[:, LCHUNK - 1, :], scalar=1.0, in1=endrep,
                            op0=ALU.mult, op1=ALU.mult,
                            accum_out=se_acc[:, 1:2],
                        )

                    # ---- transition-count gram matmuls (PSUM accumulate) ----
                    # C[i,j] += sum_b onehot_l[b,i] * onehot_{l+1}[b,j]
                    if c == 0:
                        gram = ps_gram.tile([T, T], F32, name="gram")
                    for l in (range(LCHUNK) if not SKIP_NUM else []):
                        gl = c * LCHUNK + l
                        if gl >= L - 1:
                            continue
                        lhsT = mk[:, l, :]
                        if l + 1 < LCHUNK:
                            rhs = mk[:, l + 1, :]
                        else:
                            rhs = None  # handled by next chunk's l=0 vs prev
                        if rhs is not None:
                            nc.tensor.matmul(
                                out=gram, lhsT=lhsT, rhs=rhs,
                                start=(gl == 0), stop=(gl == L - 2),
                                skip_group_check=True,
                            )
                    if c > 0 and not SKIP_NUM:
                        # boundary pair (prev chunk last l, this chunk l=0)
                        nc.tensor.matmul(
                            out=gram, lhsT=mask_tiles[c - 1][:, LCHUNK - 1, :],
                            rhs=mk[:, 0, :],
                            start=False, stop=False,
                            skip_group_check=True,
                        )

                    # ---- transpose em -> psum, exp-drain -> x pages (bf16) ----
                    # blocks of 4 l (32-padded t): in [128b, (4l,32t)] -> out
                    # [(4l*32t), 128b]
                    for blk in range(LCHUNK // 4):
                        l0 = c * LCHUNK + blk * 4
                        src = bass.AP(
                            tensor=em_t.tensor,
                            offset=em_t.offset + blk * 4 * TPAD,
                            ap=[em_t.ap[0], [1, 4 * TPAD]],
                        )
                        pg = l0 // XPAGE_L
                        col = ((l0 % XPAGE_L) // 4) * 128
                        if (l0 % XPAGE_L) == 0:
                            ps_x = ps_tr.tile([128, (XPAGE_L // 4) * 128], F32,
                                              tag="psx")
                        nc.tensor.transpose(
                            out=ps_x[:, col:col + 128], in_=src, identity=ident,
                        )
                        if (l0 % XPAGE_L) == XPAGE_L - 4:
                            nc.scalar.activation(
                                out=xpages[pg], in_=ps_x, func=ACTF.Exp,
                            )

                # ---- transition score: counts . trans ----
                tacc = small.tile([T, 1], F32, tag="tacc")
                if SKIP_NUM:
                    nc.vector.memset(tacc, 0.0)
                else:
                    nc.vector.scalar_tensor_tensor(
                    out=small.tile([T, T], F32, tag="tscr", name="tscr"),
                    in0=gram, scalar=1.0, in1=trans,
                    op0=ALU.mult, op1=ALU.mult,
                    accum_out=tacc,
                )

                # ---- masksum ----
                msum = small.tile([BLOC, 1], F32, tag="msum")
                nc.vector.tensor_reduce(out=msum, in_=mask_sb,
                                        axis=mybir.AxisListType.XYZW, op=ALU.add)

                # ================= forward scan =================
                TRW = 128 // N_TRACKS
                p = state.tile([T, 128], BF16)
                # p0 = x_0 * exp(start)
                nc.vector.tensor_scalar(
                    out=p, in0=x_slice(0, 0, 128), scalar1=estart, scalar2=None,
                    op0=ALU.mult,
                )
                if DEBUG:
                    pdbg = singles.tile([T, 512], F32)
                    nc.vector.tensor_copy(out=pdbg[:, 0:128], in_=p)
                C_idx = [0] * N_TRACKS
                for l in (range(1, L) if not SKIP_SCAN else []):
                    for tr in range(N_TRACKS):
                        c0, c1 = tr * TRW, (tr + 1) * TRW
                        q = ps_q.tile([T, TRW], F32, tag="q", name="q")
                        nc.tensor.matmul(out=q, lhsT=etrans, rhs=p[:, c0:c1],
                                         start=True, stop=True)
                        nc.vector.tensor_tensor(
                            out=p[:, c0:c1], in0=q, in1=x_slice(l, c0, c1),
                            op=ALU.mult,
                        )
                        # renorm (staggered across tracks)
                        if l % RENORM_W == (RENORM_W // 2) * tr % RENORM_W and l > 0:
                            s = ps_misc.tile([1, TRW], F32, tag="misc", name="s")
                            nc.tensor.matmul(out=s, lhsT=ones21b, rhs=p[:, c0:c1],
                                             start=True, stop=True)
                            r = small.tile([1, TRW], F32, tag="r", name="r")
                            nc.vector.reciprocal(out=r, in_=s)
                            rb = ps_misc.tile([T, TRW], F32, tag="misc", name="rb")
                            nc.tensor.matmul(out=rb, lhsT=ones1x21, rhs=r,
                                             start=True, stop=True)
                            nc.vector.tensor_tensor(out=p[:, c0:c1], in0=p[:, c0:c1],
                                                    in1=rb, op=ALU.mult)
                            # ln(s) into the deferred log buffer
                            ev = C_idx[tr]
                            C_idx[tr] += 1
                            # ACT Ln is only exact for inputs < 2^64: feed
                            # s * 2^-40 and add the 40*ln2 back on the host
                            nc.scalar.activation(
                                out=lnbuf[:, c0:c1, ev], in_=s, func=ACTF.Ln,
                                scale=2.0 ** -40,
                            )
                    if DEBUG and l <= 3:
                        nc.vector.tensor_copy(out=pdbg[:, 128 * l:128 * (l + 1)],
                                              in_=p)

                if DEBUG:
                    xcp = singles.tile([128, 512], F32)
                    nc.vector.tensor_copy(out=xcp, in_=xpages[0])
                    nc.sync.dma_start(out=xdbg_d, in_=xcp)
                    nc.sync.dma_start(out=pdbg_d, in_=pdbg)

            # ---- finalize denominator ----
            pf = small.tile([T, 128], F32, tag="pf")
            nc.vector.tensor_scalar(out=pf, in0=p, scalar1=eend, scalar2=None,
                                    op0=ALU.mult)
            zf = ps_misc.tile([1, 128], F32, tag="misc", name="zf")
            nc.tensor.matmul(out=zf, lhsT=ones21f, rhs=pf, start=True, stop=True)
            lnz = small.tile([1, 128], F32, tag="lnz")
            nc.scalar.activation(out=lnz, in_=zf, func=ACTF.Ln,
                                 scale=2.0 ** -40)
            # pad unused renorm-event slots with zeros
            nev = L // RENORM_W
            for tr in range(N_TRACKS):
                c0, c1 = tr * TRW, (tr + 1) * TRW
                if C_idx[tr] < nev:
                    nc.vector.memset(lnbuf[:, c0:c1, C_idx[tr]:nev], 0.0)
            csum = small.tile([1, 128], F32, tag="csum")
            nc.vector.tensor_reduce(out=csum, in_=lnbuf,
                                    axis=mybir.AxisListType.X, op=ALU.add)
            denomv = small.tile([1, 128], F32, tag="denomv")
            nc.vector.tensor_tensor(out=denomv, in0=lnz, in1=csum, op=ALU.add)
            dsum = small.tile([1, 1], F32, tag="dsum")
            nc.vector.tensor_reduce(out=dsum, in_=denomv,
                                    axis=mybir.AxisListType.XYZW, op=ALU.add)
            if DEBUG:
                ddbg = singles.tile([1, 8448], F32)
                nc.vector.tensor_copy(out=ddbg[:, 0:8192],
                                      in_=lnbuf.rearrange("p a b -> p (a b)"))
                nc.vector.tensor_copy(out=ddbg[:, 8192:8320], in_=lnz)
                nc.vector.tensor_copy(out=ddbg[:, 8320:8448], in_=csum)
                nc.sync.dma_start(out=ddbg_d, in_=ddbg)

            # ---- gather partials: [128, 4] -> ones-matmul -> [1, 4] ----
            parts = small.tile([BLOC, 4], F32, tag="parts")
            nc.vector.tensor_reduce(out=parts[:, 0:1], in_=em_acc,
                                    axis=mybir.AxisListType.XYZW, op=ALU.add)
            nc.vector.tensor_reduce(out=parts[:, 1:2], in_=se_acc,
                                    axis=mybir.AxisListType.XYZW, op=ALU.add)
            nc.vector.tensor_copy(out=parts[:, 2:3], in_=msum)
            nc.vector.memset(parts[:, 3:4], 0.0)
            psum4 = ps_misc.tile([1, 4], F32, tag="misc", name="psum4")
            nc.tensor.matmul(out=psum4, lhsT=ones128, rhs=parts,
                             start=True, stop=True)
            tsum = ps_misc.tile([1, 1], F32, tag="misc", name="tsum")
            nc.tensor.matmul(out=tsum, lhsT=ones21f, rhs=tacc,
                             start=True, stop=True)

            nc.vector.tensor_copy(out=out_sb[:, 0:4], in_=psum4)
            nc.vector.tensor_copy(out=out_sb[:, 4:5], in_=tsum)
            nc.vector.tensor_copy(out=out_sb[:, 5:6], in_=dsum)
            nc.sync.dma_start(out=out_d, in_=out_sb)

    return nc


_NC_CACHE = None


def _get_nc():
    global _NC_CACHE
    if _NC_CACHE is None:
        nc = bacc.Bacc("TRN2", target_bir_lowering=False, debug=False,
                       enable_asserts=False, num_devices=N_CORES)
        _build(nc)
        nc.compile()
        _NC_CACHE = nc
    return _NC_CACHE


def kernel(emissions, tags, mask, start_transitions, end_transitions,
           transitions):
    em = np.ascontiguousarray(np.asarray(emissions, dtype=np.float32))
    tg = np.ascontiguousarray(np.asarray(tags).astype(np.int32))
    mk = np.ascontiguousarray(np.asarray(mask).astype(np.uint8))
    start = np.asarray(start_transitions, dtype=np.float32)
    end = np.asarray(end_transitions, dtype=np.float32)
    trans = np.ascontiguousarray(np.asarray(transitions, dtype=np.float32))

    etrans = np.exp(trans.astype(np.float64)).astype(ml_dtypes.bfloat16)
    estart = np.exp(start.astype(np.float64)).astype(np.float32)
    eend = np.exp(end.astype(np.float64)).astype(np.float32)

    def pack_blob(tg_sh, mk_sh):
        blob = np.zeros((128, BLOB_BYTES), np.uint8)

        def put(off, arr2d):
            a = np.ascontiguousarray(arr2d)
            b = a.view(np.uint8).reshape(a.shape[0], -1)
            blob[:b.shape[0], off:off + b.shape[1]] = b

        put(OFF_TRANS, trans)
        put(OFF_STARTREP, np.broadcast_to(start, (128, T)))
        put(OFF_ENDREP, np.broadcast_to(end, (128, T)))
        put(OFF_ESTART, np.pad(estart.reshape(T, 1), ((0, 107), (0, 0))))
        put(OFF_EEND, np.pad(eend.reshape(T, 1), ((0, 107), (0, 0))))
        put(OFF_ONESF, np.ones((128, 1), np.float32))
        put(OFF_ONESROW, np.ones((128, T), np.float32))
        put(OFF_ETRANS, etrans)
        put(OFF_ONESB, np.ones((128, 1), ml_dtypes.bfloat16))
        put(OFF_IOTA, np.broadcast_to(np.arange(T, dtype=np.int32), (128, T)))
        put(OFF_TAGS, tg_sh)
        put(OFF_MASK, mk_sh)
        put(OFF_IDENT, np.eye(128, dtype=np.float32))
        return blob

    in_maps = []
    for c in range(N_CORES):
        sl = slice(c * BLOC, (c + 1) * BLOC)
        in_maps.append(dict(em=em[sl], blob=pack_blob(tg[sl], mk[sl])))

    nc = _get_nc()
    global _last_in_maps, _last_results
    _last_in_maps = in_maps
    res = run_bass_kernel_spmd(nc, in_maps, core_ids=list(range(N_CORES)))
    _last_results = res.results

    score = 0.0
    denom = 0.0
    masksum = 0.0
    # every Ln on device was fed s * 2^-40; add the offsets back:
    # per core, track0 columns have 63 renorm events, track1 have 64,
    # plus one final logsumexp per sequence column.
    ln_corr = (64 * 63 + 64 * 64 + 128) * 40.0 * np.log(2.0)
    for r in res.results:
        o = r["out"].astype(np.float64).ravel()
        score += o[0] + o[1] + o[4]   # emission + start/end + transition
        denom += o[5] + ln_corr
        masksum += o[2]
    return np.float32((score - denom) / masksum)

